# revision 9
# baseline (speedup 1.0000x reference)
"""Matrix NMS (SOLOv2 gaussian decay) on 8 TRN2 NeuronCores — column-stripe v2.

Strategy: shard the pixel (h*w=40960) contraction dim across the 8 cores
for the Gram matmul (fp8 DoubleRow, upper block-rows), but shard the
EPILOGUE by column stripes: core c owns IoU columns [128c, 128c+128).
The partial-Gram combine is an AllToAll whose dest-c chunk is the column
stripe c, split into three row-range chunks (blocks 0-2 / 3-5 / 6-7) so
the first two ship while later block-rows are still computing.  With
column stripes BOTH cross-candidate reductions are core-local:
compensate (column max) reduces over rows held in-core, and the final
decay max only needs the full compensate^2 vector — one 512B-per-core
AllGather — instead of the two serialized ReduceScatters of the
row-stripe design.  Drains go PSUM -> int16 SBUF on the scalar/ACT
engine (DVE stays free for the receive-side tree-sum + IoU math, which
pipelines per chunk behind the AllToAlls); the d^2 chunk transposes run
on the PE after the Gram.  Every core ends with the decayed scores for
its own 128 candidates; the host concatenates the 8 slices.
"""
import sys

import numpy as np

for _p in ("/opt/trn_rl_repo",):
    if _p not in sys.path:
        sys.path.insert(0, _p)

from concourse import bacc, bass, mybir, tile
from concourse import bass_utils

N = 1024           # candidates
HWPIX = 160 * 256  # 40960 pixels
W = 8              # cores
KC = HWPIX // W    # 5120 pixel-slice per core
KT = KC // 128     # 40 k-tiles of 128
GRP = 4            # k-tiles per resident SBUF group
RB = 128           # stripe width == candidates per core
NP = KT // 2       # 20 k-tile pairs (DoubleRow)
SIGMA = 2.0
BOUNDS = (6, 8)  # block-row boundaries of the AllToAll chunks: per-collective
# fixed cost is ~5-7us and sub-512KB chunks only reach ~26GB/s, so two big
# chunks beat finer pipelining; the small tail chunk still overlaps the
# receive-side math of the first

F32 = mybir.dt.float32
FP8 = mybir.dt.float8e4  # e4m3: exact for 0/1 mask values
I16 = mybir.dt.int16


def build_nc(variant="full"):
    # variant: "full" = real kernel; "nocc" = collectives replaced by local
    # DMA copies (wrong math, identical local compute/DMA — timing/sim only)
    nc = bacc.Bacc(
        "TRN2", target_bir_lowering=False, debug=False,
        num_devices=W if variant == "full" else 1,
    )

    xT = nc.dram_tensor("xT", [128, KT * N], FP8, kind="ExternalInput")
    sjsc_h = nc.dram_tensor("sjsc", [128, N], F32, kind="ExternalInput")
    scores_h = nc.dram_tensor("scores", [1, RB], F32, kind="ExternalInput")
    ident_h = nc.dram_tensor("ident", [128, 128], F32, kind="ExternalInput")
    out_h = nc.dram_tensor("out", [1, RB], F32, kind="ExternalOutput")

    RG = [list(range(W))]
    CH = [((0,) + BOUNDS)[i] for i in range(len(BOUNDS))]
    CH = list(zip((0,) + BOUNDS[:-1], BOUNDS))  # [(0,3),(3,6),(6,8)]

    with tile.TileContext(nc) as tc:
        with (
            tc.tile_pool(name="dram", bufs=1, space="DRAM") as dramp,
            tc.tile_pool(name="xp", bufs=1) as xp,
            tc.tile_pool(name="pg", bufs=4, space="PSUM") as pgp,
            tc.tile_pool(name="gb", bufs=2) as gbp,
            tc.tile_pool(name="rx", bufs=1) as rxp,
            tc.tile_pool(name="sc", bufs=1) as scp,
            tc.tile_pool(name="epi", bufs=1) as ep,
        ):
            # AllToAll buffers: chunk x holds block-rows [b0, b1) of every
            # column stripe; dest-c chunk = rows of stripe c.
            cc_h = [
                dramp.tile([W * 128 * (b1 - b0), RB], I16,
                           tag=f"cc{x}", name=f"cc{x}")
                for x, (b0, b1) in enumerate(CH)
            ]
            a2a_h = [
                dramp.tile([W * 128 * (b1 - b0), RB], I16,
                           tag=f"a2a{x}", name=f"a2a{x}")
                for x, (b0, b1) in enumerate(CH)
            ]
            ag_in = dramp.tile([1, RB], F32, tag="ag_in")
            ag_out = dramp.tile([1, N], F32, tag="ag_out")

            # ---- phase 1: fp8 x slice straight into SBUF, first on both
            # bulk queues so the PE can start right away; the first group is
            # split in half so the first matmul's k-pair lands sooner
            GSZ = [2, 2] + [GRP] * ((KT - 4) // GRP)  # k-tiles per group
            xg = [xp.tile([128, GSZ[g], N], FP8, tag=f"x{g}", name=f"xg{g}")
                  for g in range(len(GSZ))]
            goff = [0]
            for g in GSZ:
                goff.append(goff[-1] + g)
            for g in range(len(GSZ)):
                eng = nc.sync if g % 2 == 0 else nc.scalar
                eng.dma_start(
                    xg[g][:], xT[:, goff[g] * N : goff[g + 1] * N]
                )
            # constants on the gpsimd queue (idle until the first AllToAll
            # trigger); sjsc behind the even x groups on the sync queue to
            # keep it off the critical first x tiles
            ident = scp.tile([128, 128], F32, tag="ident")
            nc.gpsimd.dma_start(ident[:], ident_h[:])
            scores = scp.tile([1, RB], F32, tag="scores")
            nc.gpsimd.dma_start(scores[:], scores_h[:])
            sjsc = scp.tile([128, N], F32, tag="sjsc")
            nc.sync.dma_start(sjsc[:], sjsc_h[:])
            ones_r = scp.tile([1, 128], F32, tag="ones_r")
            nc.vector.memset(ones_r[:], 1.0)

            def xpair(q, c0, c1):
                t = 2 * q
                g = next(i for i in range(len(GSZ)) if goff[i + 1] > t)
                j = t - goff[g]
                return xg[g][:, j : j + 2, c0:c1]

            def gram_pair(pg, a, q):
                # 512-col chunks: ISA limit on moving elements per matmul
                wdt = N - a * 128
                lhsT = xpair(q, a * 128, (a + 1) * 128)
                for off in range(0, wdt, 512):
                    cw = min(512, wdt - off)
                    nc.tensor.matmul(
                        pg[:, off : off + cw],
                        lhsT,
                        xpair(q, a * 128 + off, a * 128 + off + cw),
                        start=(q == 0),
                        stop=(q == NP - 1),
                        perf_mode=mybir.MatmulPerfMode.DoubleRow,
                    )

            def drain(a, pg):
                """PSUM block-row a -> int16 tiles of the column-stripe
                AllToAll chunks (stripes c >= a; left stripes stay garbage,
                masked by the receiver's sjsc table)."""
                wdt = N - a * 128
                gb16 = gbp.tile([128, wdt], I16, tag="gb16", name=f"gb{a}")
                # ACT engine only: DVE stays free for the receive-side work
                nc.scalar.activation(
                    gb16[:], pg[:, :wdt], mybir.ActivationFunctionType.Copy
                )
                x = next(i for i, (b0, b1) in enumerate(CH) if b0 <= a < b1)
                b0, b1 = CH[x]
                b = a - b0
                dst = cc_h[x][:].rearrange(
                    "(c g p) j -> p g c j", c=W, g=b1 - b0, p=128
                )[:, b, a:W, :]
                src = gb16[:].rearrange("p (c j) -> p c j", c=W - a)
                eng = (nc.sync, nc.scalar)[a % 2]
                eng.dma_start(dst, src)

            def a2a_chunk(x):
                if variant == "full":
                    nc.gpsimd.collective_compute(
                        "AllToAll",
                        mybir.AluOpType.bypass,
                        replica_groups=RG,
                        ins=[cc_h[x][:].opt()],
                        outs=[a2a_h[x][:].opt()],
                    )
                else:
                    nc.sync.dma_start(a2a_h[x][:], cc_h[x][:])

            # ---- phase 2: Gram upper block-rows, block-sequential; each
            # AllToAll chunk fires as soon as its last block-row is drained
            for a in range(W):
                pg = pgp.tile([128, N - a * 128], F32, tag="pg", name=f"pgS{a}")
                for q in range(NP):
                    gram_pair(pg, a, q)
                drain(a, pg)
                for x, (b0, b1) in enumerate(CH):
                    if a == b1 - 1:
                        a2a_chunk(x)

            # ---- receive side: per chunk, tree-sum the 8 partials of MY
            # column stripe and run the row-oriented IoU math (all DVE)
            summ = ep.tile([128, W, 128], F32, tag="summ")
            d2 = ep.tile([128, W, 128], F32, tag="d2")

            def tree_epi(x):
                b0, b1 = CH[x]
                nb = b1 - b0
                lt = rxp.tile([128, W, nb, 128], I16, tag=f"lt{x}", name=f"lt{x}")
                src = a2a_h[x][:].rearrange(
                    "(s g p) j -> p s g j", s=W, g=nb, p=128
                )
                nc.sync.dma_start(lt[:, 0:4, :, :], src[:, 0:4, :, :])
                nc.scalar.dma_start(lt[:, 4:8, :, :], src[:, 4:8, :, :])
                a4 = ep.tile([128, 4, nb, 128], I16, tag=f"a4{x}")
                for s in range(4):
                    nc.vector.tensor_add(
                        a4[:, s, :, :], lt[:, 2 * s, :, :], lt[:, 2 * s + 1, :, :]
                    )
                b2 = ep.tile([128, 2, nb, 128], I16, tag=f"b2{x}")
                nc.vector.tensor_add(b2[:, 0, :, :], a4[:, 0, :, :], a4[:, 1, :, :])
                nc.vector.tensor_add(b2[:, 1, :, :], a4[:, 2, :, :], a4[:, 3, :, :])
                nc.vector.tensor_add(summ[:, b0:b1, :], b2[:, 0, :, :], b2[:, 1, :, :])
                # un = sjsc - inter; masked entries have sjsc=1e30 so d ~ 0
                un = ep.tile([128, nb, 128], F32, tag=f"un{x}")
                nc.vector.tensor_tensor(
                    un[:], sjsc[:, b0 * 128 : b1 * 128].rearrange(
                        "p (g j) -> p g j", g=nb
                    ),
                    summ[:, b0:b1, :], op=mybir.AluOpType.subtract,
                )
                nc.vector.tensor_scalar(
                    un[:], un[:], 1.0, None, op0=mybir.AluOpType.max
                )
                rec = ep.tile([128, nb, 128], F32, tag=f"rec{x}")
                nc.vector.reciprocal_approx_fast(rec[:], un[:])
                nc.vector.tensor_mul(d2[:, b0:b1, :], summ[:, b0:b1, :], rec[:])
                nc.vector.tensor_mul(d2[:, b0:b1, :], d2[:, b0:b1, :], d2[:, b0:b1, :])

            for x in range(len(CH)):
                tree_epi(x)

            # ---- d^2 chunk transposes on the (now idle) PE into ONE PSUM
            # tile; c^2 = column max is then a free-dim reduce read straight
            # from PSUM, fully core-local
            tp = pgp.tile([128, W, 128], F32, tag="pg", name="tp")
            for g in range(W):
                nc.tensor.transpose(tp[:, g, :], d2[:, g, :], ident[:])
            mx8 = ep.tile([128, W], F32, tag="mx8")
            nc.vector.tensor_reduce(
                mx8[:], tp[:], axis=mybir.AxisListType.X, op=mybir.AluOpType.max
            )
            c2l = ep.tile([128, 1], F32, tag="c2l")
            nc.vector.tensor_reduce(
                c2l[:], mx8[:], axis=mybir.AxisListType.X, op=mybir.AluOpType.max
            )
            c2ps = pgp.tile([1, 128], F32, tag="pg", name="c2ps")
            nc.tensor.transpose(c2ps[:], c2l[:], ident[:])
            c2row = ep.tile([1, 128], F32, tag="c2row")
            nc.vector.tensor_copy(c2row[:], c2ps[:])
            nc.sync.dma_start(ag_in[:], c2row[:])

            # ---- one tiny AllGather: full compensate^2 vector (4KB)
            if variant == "full":
                nc.gpsimd.collective_compute(
                    "AllGather",
                    mybir.AluOpType.bypass,
                    replica_groups=RG,
                    ins=[ag_in[:].opt()],
                    outs=[ag_out[:].opt()],
                )
            else:
                for c in range(W):
                    nc.sync.dma_start(ag_out[:, c * RB : (c + 1) * RB], ag_in[:])
            crow = ep.tile([1, N], F32, tag="crow")
            nc.sync.dma_start(crow[:], ag_out[:])
            # broadcast c^2 across partitions on gpsimd; free layout (g p)
            # equals global row order, matching tp's [j, g, p]
            cbs = ep.tile([128, W, 128], F32, tag="cbs")
            nc.gpsimd.partition_broadcast(cbs[:], crow[:])
            # M = max over rows of (d^2 - c^2), then decay = exp(-sigma*M)
            fTm = ep.tile([128, W, 128], F32, tag="fTm")
            nc.vector.tensor_tensor(
                fTm[:], tp[:], cbs[:], op=mybir.AluOpType.subtract
            )
            mxf = ep.tile([128, W], F32, tag="mxf")
            nc.vector.tensor_reduce(
                mxf[:], fTm[:], axis=mybir.AxisListType.X, op=mybir.AluOpType.max
            )
            mloc = ep.tile([128, 1], F32, tag="mloc")
            nc.vector.tensor_reduce(
                mloc[:], mxf[:], axis=mybir.AxisListType.X, op=mybir.AluOpType.max
            )
            mps = pgp.tile([1, 128], F32, tag="pg", name="mps")
            nc.tensor.transpose(mps[:], mloc[:], ident[:])
            coeff = ep.tile([1, RB], F32, tag="coeff")
            nc.scalar.activation(
                coeff[:], mps[:], mybir.ActivationFunctionType.Exp, scale=-SIGMA
            )
            outsb = ep.tile([1, RB], F32, tag="outsb")
            nc.vector.tensor_mul(outsb[:], coeff[:], scores[:])
            nc.scalar.dma_start(out_h[:], outsb[:])

    nc.compile()
    return nc


_NC_CACHE = {}


def _get_nc(variant="full"):
    if variant not in _NC_CACHE:
        _NC_CACHE[variant] = build_nc(variant)
    return _NC_CACHE[variant]


def make_in_maps(seg_masks, cate_labels, cate_scores):
    import ml_dtypes

    flat = np.asarray(seg_masks, dtype=np.float32).reshape(N, -1)
    labels = np.asarray(cate_labels)
    scores = np.asarray(cate_scores, dtype=np.float32)
    areas = flat.sum(axis=1)  # exact integers in f32
    xTfull = np.ascontiguousarray(flat.T)  # (40960, 1024)
    ident = np.eye(128, dtype=np.float32)
    r = np.arange(N)
    in_maps = []
    for c in range(W):
        cols = np.arange(c * RB, (c + 1) * RB)
        valid = (r[:, None] < cols[None, :]) & (
            labels[:, None] == labels[cols][None, :]
        )
        # masked entries get a huge union so d = inter/union underflows to ~0
        sj = np.where(
            valid, areas[:, None] + areas[cols][None, :], 1e30
        ).astype(np.float32)  # [1024 rows, 128 stripe cols]
        # SBUF layout [p, (g j)]: partition p holds rows {p, 128+p, ...}
        sjsc = np.ascontiguousarray(
            sj.reshape(8, 128, RB).transpose(1, 0, 2).reshape(128, 8 * RB)
        )
        in_maps.append(
            {
                # partition-major: row p holds k-rows {p, 128+p, ...} of this
                # core's slice; host casts to fp8 (exact for 0/1 masks)
                "xT": np.ascontiguousarray(
                    xTfull[c * KC : (c + 1) * KC]
                    .reshape(KT, 128, N)
                    .transpose(1, 0, 2)
                ).reshape(128, KT * N).astype(ml_dtypes.float8_e4m3fn),
                "sjsc": sjsc,
                "scores": np.ascontiguousarray(scores[cols].reshape(1, RB)),
                "ident": ident,
            }
        )
    return in_maps


def run_device(in_maps, trace=False, trace_cores=None):
    nc = _get_nc()
    res = bass_utils.run_bass_kernel_spmd(
        nc, in_maps, core_ids=list(range(W)), trace=trace,
        trace_cores=trace_cores,
    )
    return res


def kernel(seg_masks, cate_labels, cate_scores):
    in_maps = make_in_maps(seg_masks, cate_labels, cate_scores)
    res = None
    for attempt in range(3):
        try:
            res = run_device(in_maps)
            break
        except Exception:
            # transient NRT_EXEC_UNIT_UNRECOVERABLE / tunnel hiccups: the
            # device usually recovers after a short pause
            if attempt == 2:
                raise
            import time

            time.sleep(30)
    outs = [np.asarray(res.results[c]["out"]).reshape(RB) for c in range(W)]
    return np.concatenate(outs).astype(np.float32)
